# revision 28
# baseline (speedup 1.0000x reference)
"""Bahdanau (additive) attention kernel for Trainium2, 8 NeuronCores.

Problem shapes: inp (B=4, T=128, D=512), context (B=4, S=512, D=512).
  wq   = inp @ Wq.T + bq                      (B,T,D)
  uh   = context @ Wc.T                       (B,S,D)
  align= einsum('btsd,d->bts', tanh(wq[:,:,None,:]+uh[:,None,:,:]), v)
  a    = softmax(align, -1)                   (B,T,S)
  c    = einsum('bts,bsd->btd', a, context)
  attn = concat([c, inp], -1) @ Wout.T + bout (B,T,D)
Returns (attn, a).

Algorithm: the O(T*S*D) tanh stream is replaced by a separable sinusoid
expansion.  tanh(x) ~ sum_j c_j sin(n_j w x), odd harmonics n_j = 2j+1
(the Neumann-reflected periodic extension of tanh on [-L,L] has only
odd terms).  With a = wq, b = uh:

  align[t,s] ~= sum_{d,j} [c_j sin(n_j w a)] [v_d cos(n_j w b)]
              + [c_j cos(n_j w a)] [v_d sin(n_j w b)]

i.e. one PE matmul with contraction (d, j, phase) = 2*J*D plus
O((T+S)*D*J) trig features.  Feature generation: ACT computes the
n=1 seeds (sin at w/2 and w; cos and E2 = 2cos(2w x) via Square so
every ACT argument stays inside the table range [-pi,pi]); higher
harmonics come from fp16 Chebyshev ladders
  X_{n+2} = E2 * X_n - X_{n-2}
run as paired [sin||cos] tensor_tensor ops on DVE (B side, split in
two chunk-groups so the ladder starts as soon as the first half of the
seeds lands) and DVE+Pool (A side).  v_d rides the (linear) B ladder
seeds; c_j is applied to the raw A harmonics as one wide tensor_scalar
per feature.  All weight/activation operands are pre-swizzled on the
host to [128, C*F] so every DMA is a plain contiguous 2D transfer.

Sharding: 8 cores = (batch b, source-half sh); each core computes its
[T=128, SH=256] block of unnormalized p = exp(align), partial row sums
sig, partial output V = p16 @ M (M = ctx_half @ WoutC), and
I = inp@WoutI + bout.  The host finishes the cross-shard reduction at
gather time: attn = (V0+V1)/(sig0+sig1) + I, align = p/(sig0+sig1).
"""

import numpy as np

import concourse.bacc as bacc
import concourse.tile as tile
from concourse import mybir
from concourse.bass import ds, ts
from concourse.bass_utils import run_bass_kernel_spmd
from concourse.masks import make_identity

F32 = mybir.dt.float32
F16 = mybir.dt.float16
ALU = mybir.AluOpType

B, T, S, D = 4, 128, 512, 512
SH = S // 2  # source positions per core
N_CORES = 8
NCH = D // 128  # partition chunks of the model dim

J = 6
L = 5.8
OM = float(np.pi / (2 * L))
CS = [1.23639529, 0.32902321, 0.13184508, 0.05500646,
      0.02290538, 0.00915409][:J]

_NC_CACHE = {}


def _build_nc():
    nc = bacc.Bacc("TRN2", target_bir_lowering=False, debug=False, num_devices=N_CORES)

    ctxT = nc.dram_tensor("ctxT", [128, NCH * SH], F16, kind="ExternalInput")
    wcT = nc.dram_tensor("wcT", [128, NCH * D], F16, kind="ExternalInput")
    wqT = nc.dram_tensor("wqT", [128, NCH * D], F16, kind="ExternalInput")
    inpT = nc.dram_tensor("inpT", [128, NCH * T], F16, kind="ExternalInput")
    woutT = nc.dram_tensor("woutT", [128, 2 * NCH * D], F16, kind="ExternalInput")
    bq = nc.dram_tensor("bq", [128, NCH], F32, kind="ExternalInput")
    v = nc.dram_tensor("v", [128, NCH], F32, kind="ExternalInput")
    bout = nc.dram_tensor("bout", [D], F32, kind="ExternalInput")
    p_out = nc.dram_tensor("p_out", [T, SH], F32, kind="ExternalOutput")
    sig = nc.dram_tensor("sig", [T, 1], F32, kind="ExternalOutput")
    V_out = nc.dram_tensor("V_out", [T, D], F32, kind="ExternalOutput")
    I_out = nc.dram_tensor("I_out", [T, D], F32, kind="ExternalOutput")

    with tile.TileContext(nc) as tc:
        _emit(nc, tc, ctxT, wcT, wqT, inpT, woutT, bq, v, bout,
              p_out, sig, V_out, I_out)
    nc.compile()
    return nc


def _emit(nc, tc, ctxT, wcT, wqT, inpT, woutT, bq, v, bout,
          p_out, sig, V_out, I_out):
    Sin = mybir.ActivationFunctionType.Sin
    Sq = mybir.ActivationFunctionType.Square
    Exp = mybir.ActivationFunctionType.Exp
    AT = NCH * T    # 512: A-side wide free size
    BT = NCH * SH   # 1024: B-side wide free size
    with (
        tc.tile_pool(name="persist", bufs=1) as P,
        tc.tile_pool(name="uh_ps", bufs=1, space="PSUM") as uh_pool,
        tc.tile_pool(name="wq_ps", bufs=1, space="PSUM") as wq_pool,
        tc.tile_pool(name="al_ps", bufs=1, space="PSUM") as al_pool,
        tc.tile_pool(name="ep_ps", bufs=2, space="PSUM") as ep_pool,
    ):
        # ---- t=0: table preload, PE warmup, DMAs -------------------------
        dumt = P.tile([1, 16], F16, name="dumt", tag="dumt")
        nc.vector.memset(dumt, 0.0)
        dumo = P.tile([1, 16], F16, name="dumo", tag="dumo")
        # first ACT op: loads the trig table set while DMAs run
        nc.scalar.activation(dumo, dumt, Sin)

        warm_sb = P.tile([128, SH], F16, name="warm_sb", tag="warm_sb")
        nc.vector.memset(warm_sb, 0.0)
        warm_ps = ep_pool.tile([128, SH], F32, name="warm_ps", tag="ep")
        for r in range(4):
            nc.tensor.matmul(warm_ps[0:64, :], lhsT=warm_sb[:, 0:64], rhs=warm_sb,
                             start=(r == 0), stop=(r == 3))

        def filler(n=1):
            # keep the PE HAM clock warm during feature-ladder stalls
            for r in range(n):
                nc.tensor.matmul(warm_ps[0:64, 0:128], lhsT=warm_sb[:, 0:64],
                                 rhs=warm_sb[:, 0:128], start=True, stop=True)

        def load_wide(name, dram, engine=None):
            # host pre-swizzles to [128, C*F]: plain contiguous 2D DMA
            t = P.tile([128, dram.shape[1]], F16, name=name, tag=name)
            eng = engine or nc.sync
            eng.dma_start(out=t, in_=dram.ap())
            return t

        v_sb = P.tile([128, NCH], F32, name="v_sb", tag="v_sb")
        nc.sync.dma_start(out=v_sb, in_=v.ap())
        bq_sb = P.tile([128, NCH], F32, name="bq_sb", tag="bq_sb")
        nc.sync.dma_start(out=bq_sb, in_=bq.ap())
        ctxT_all = load_wide("ctxT_all", ctxT)          # [128, 4*SH]
        wcT_all = load_wide("wcT_all", wcT, nc.gpsimd)  # [128, 4*D]
        wqT_all = load_wide("wqT_all", wqT, nc.gpsimd)  # [128, 4*D]
        inpT_all = load_wide("inpT_all", inpT, nc.sync)  # [128, 4*T]

        ctxT_sb = [ctxT_all[:, ds(SH * i, SH)] for i in range(NCH)]
        wcT_sb = [wcT_all[:, ds(D * i, D)] for i in range(NCH)]
        wqT_sb = [wqT_all[:, ds(D * i, D)] for i in range(NCH)]
        inpT_sb = [inpT_all[:, ds(T * i, T)] for i in range(NCH)]

        ident = P.tile([128, 128], F16, name="ident", tag="ident")
        make_identity(nc, ident)
        ones_sb = P.tile([1, T], F16, name="ones_sb", tag="ones_sb")
        nc.gpsimd.memset(ones_sb, 1.0)
        # sin(w (wq + bq)) = Sin(scale=w, bias=w*bq) on the raw wq PSUM
        bqw2 = P.tile([128, NCH], F32, name="bqw2", tag="bqw2")
        nc.vector.tensor_scalar_mul(bqw2, bq_sb, OM / 2)
        bqw = P.tile([128, NCH], F32, name="bqw", tag="bqw")
        nc.vector.tensor_scalar_mul(bqw, bq_sb, OM)

        # ---- B-side: uh matmuls + per-chunk seed pipeline ----------------
        # seeds: sh = sin(w/2 uh), s1 = sin(w uh) straight from PSUM;
        # cos1 = 1-2 sh^2, E2 = 2-4 s1^2 (Square + tensor_scalar).
        uh_wide = uh_pool.tile([128, BT], F32, name="uh_wide", tag="uh")
        uh_ps = [uh_wide[:, ts(k, SH)] for k in range(NCH)]
        shB = P.tile([128, BT], F16, name="shB", tag="shB")
        s1Br = P.tile([128, BT], F16, name="s1Br", tag="s1Br")
        qB = P.tile([128, BT], F16, name="qB", tag="qB")
        qB2 = P.tile([128, BT], F16, name="qB2", tag="qB2")
        c1Br = P.tile([128, BT], F16, name="c1Br", tag="c1Br")
        E2Bp = P.tile([128, 2 * BT], F16, name="E2Bp", tag="E2Bp")
        # paired [sin || cos] B feature tiles, v-scaled
        Bp = [P.tile([128, 2 * BT], F16, name=f"Bp{j}", tag=f"Bp{j}")
              for j in range(J)]

        # Bp layout is group-major: group g (chunks 2g,2g+1) occupies
        # [g*BT : g*BT+BT] as [sin(512) || cos(512)]; E2Bp matches with the
        # E2 values duplicated into both halves of each group.
        GB = BT  # 1024: bytes of one [sin||cos] group block
        def bsin(k):
            return (k // 2) * GB + (k % 2) * SH
        def bcos(k):
            return (k // 2) * GB + 2 * SH + (k % 2) * SH
        for k in range(NCH):
            for j in range(NCH):
                nc.tensor.matmul(uh_ps[k], lhsT=wcT_sb[j][:, ts(k, 128)],
                                 rhs=ctxT_sb[j], start=(j == 0), stop=(j == NCH - 1))
            cc = ts(k, SH)
            nc.scalar.activation(shB[:, cc], uh_ps[k], Sin, scale=OM / 2)
            nc.scalar.activation(s1Br[:, cc], uh_ps[k], Sin, scale=OM)
            if k % 2 == 1:
                g = ds((k - 1) * SH, 2 * SH)
                gb = (k - 1) // 2 * GB
                nc.scalar.activation(qB[:, g], shB[:, g], Sq)
                nc.scalar.activation(qB2[:, g], s1Br[:, g], Sq)
                nc.vector.tensor_scalar(c1Br[:, g], qB[:, g], -2.0, 1.0,
                                        ALU.mult, ALU.add)
                nc.vector.tensor_scalar(E2Bp[:, ds(gb, 2 * SH)], qB2[:, g],
                                        -4.0, 2.0, ALU.mult, ALU.add)
                nc.vector.tensor_scalar(E2Bp[:, ds(gb + 2 * SH, 2 * SH)],
                                        qB2[:, g], -4.0, 2.0, ALU.mult, ALU.add)
                for kk in (k - 1, k):
                    nc.gpsimd.tensor_scalar_mul(Bp[0][:, ds(bcos(kk), SH)],
                                                c1Br[:, ts(kk, SH)],
                                                v_sb[:, kk:kk + 1])
            nc.gpsimd.tensor_scalar_mul(Bp[0][:, ds(bsin(k), SH)], s1Br[:, cc],
                                        v_sb[:, k:k + 1])

        # ---- A-side: wq matmuls + seeds (PSUM + bias trick) --------------
        wq_wide = wq_pool.tile([128, AT], F32, name="wq_wide", tag="wq")
        wq_ps = [wq_wide[:, ts(k, T)] for k in range(NCH)]
        shA = P.tile([128, AT], F16, name="shA", tag="shA")
        s1Ar = P.tile([128, AT], F16, name="s1Ar", tag="s1Ar")
        qA = P.tile([128, AT], F16, name="qA", tag="qA")
        qA2 = P.tile([128, AT], F16, name="qA2", tag="qA2")
        c1Ar = P.tile([128, AT], F16, name="c1Ar", tag="c1Ar")
        # E2A duplicated [E2A || E2A] for paired A steps
        E2Ap = P.tile([128, 2 * AT], F16, name="E2Ap", tag="E2Ap")
        for k in range(NCH):
            for j in range(NCH):
                nc.tensor.matmul(wq_ps[k], lhsT=wqT_sb[j][:, ts(k, 128)],
                                 rhs=inpT_sb[j], start=(j == 0), stop=(j == NCH - 1))
            cc = ts(k, T)
            nc.scalar.activation(shA[:, cc], wq_ps[k], Sin, scale=OM / 2,
                                 bias=bqw2[:, k:k + 1])
            nc.scalar.activation(s1Ar[:, cc], wq_ps[k], Sin, scale=OM,
                                 bias=bqw[:, k:k + 1])
            nc.scalar.activation(qA[:, cc], shA[:, cc], Sq)
            nc.scalar.activation(qA2[:, cc], s1Ar[:, cc], Sq)
            nc.vector.tensor_scalar(c1Ar[:, cc], qA[:, cc], -2.0, 1.0,
                                    ALU.mult, ALU.add)
            nc.vector.tensor_scalar(E2Ap[:, cc], qA2[:, cc], -4.0, 2.0,
                                    ALU.mult, ALU.add)
            nc.vector.tensor_scalar(E2Ap[:, ds(AT + k * T, T)], qA2[:, cc],
                                    -4.0, 2.0, ALU.mult, ALU.add)

        # all Sin work is done once the A seeds above retire; preload the
        # exp table now (data-dependent on the last Square so the scheduler
        # keeps it after every Sin) so the softmax tail pays no table load
        nc.scalar.activation(dumo, qA2[0:1, AT - 16:AT], Exp)

        # raw paired A harmonic chain + c_j-scaled feature tiles
        Ar = [P.tile([128, 2 * AT], F16, name=f"Ar{j}", tag=f"Ar{j}")
              for j in range(J)]
        Ap = [P.tile([128, 2 * AT], F16, name=f"Ap{j}", tag=f"Ap{j}")
              for j in range(J)]
        nc.vector.tensor_copy(Ar[0][:, 0:AT], s1Ar)
        nc.vector.tensor_copy(Ar[0][:, AT:2 * AT], c1Ar)
        nc.vector.tensor_scalar_mul(Ap[0], Ar[0], CS[0])

        # ---- epilogue operands (loaded/computed mid-stream) --------------
        woutT_all = load_wide("woutT_all", woutT, nc.scalar)
        woutT_sb = [woutT_all[:, ds(D * i, D)] for i in range(2 * NCH)]
        bout_f32 = P.tile([1, D], F32, name="bout_f32", tag="bout_f32")
        nc.scalar.dma_start(out=bout_f32, in_=bout.ap().rearrange("(o f) -> o f", o=1))
        bout_sb = P.tile([1, D], F16, name="bout_sb", tag="bout_sb")
        nc.gpsimd.tensor_copy(bout_sb, bout_f32)

        align_ps = al_pool.tile([T, SH], F32, name="align", tag="align")

        def align_mm(j, start, stop):
            # align += As_j^T Bc_j + Ac_j^T Bs_j over the 4 d-chunks
            for k in range(NCH):
                nc.tensor.matmul(align_ps, lhsT=Ap[j][:, ts(k, T)],
                                 rhs=Bp[j][:, ds(bcos(k), SH)],
                                 start=start and k == 0, stop=False)
            for k in range(NCH):
                nc.tensor.matmul(align_ps, lhsT=Ap[j][:, ds(AT + k * T, T)],
                                 rhs=Bp[j][:, ds(bsin(k), SH)],
                                 start=False, stop=stop and k == NCH - 1)

        M_sb = P.tile([128, 2 * D], F16, name="M_sb", tag="M_sb")

        def emit_M_chunk(sc):
            # M[s, e] = sum_f ctx[s, f] Wout_c[e, f]
            ps = ep_pool.tile([128, D], F32, name=f"M{sc}", tag="ep")
            for j in range(NCH):
                nc.tensor.matmul(ps, lhsT=ctxT_all[:, ds(SH * j + 128 * sc, 128)],
                                 rhs=woutT_sb[j], start=(j == 0), stop=(j == NCH - 1))
            nc.scalar.copy(M_sb[:, ts(sc, D)], ps)

        I_sb = P.tile([T, D], F32, name="I_sb", tag="I_sb")

        def emit_I():
            ps = ep_pool.tile([T, D], F32, name="I_ps", tag="ep")
            nc.tensor.matmul(ps, lhsT=ones_sb[:, 0:T], rhs=bout_sb,
                             start=True, stop=False)
            for f in range(NCH):
                nc.tensor.matmul(ps, lhsT=inpT_sb[f], rhs=woutT_sb[NCH + f],
                                 start=False, stop=(f == NCH - 1))
            nc.scalar.copy(I_sb, ps)
            nc.scalar.dma_start(out=I_out.ap(), in_=I_sb)

        # ---- ladders + align accumulation --------------------------------
        # B (v-carried, unscaled c): X_{j} = E2B*X_{j-1} -/+ X_{j-2}
        # A (c-folded): F_j = Ea_j*F_{j-1} + (-beta_j) F_{j-2} via STT
        tmpB = [P.tile([128, 2 * BT], F16, name=f"tmpB{i}", tag=f"tmpB{i}")
                for i in range(2)]
        tmpA = [P.tile([128, 2 * AT], F16, name=f"tmpA{i}", tag=f"tmpA{i}")
                for i in range(2)]

        emit_M_chunk(0)
        emit_M_chunk(1)
        emit_I()
        align_mm(0, True, False)

        for j in range(1, J):
            tB = tmpB[j % 2]
            tA = tmpA[j % 2]
            for g in range(2):
                gb = ds(g * GB, GB)
                nc.vector.tensor_tensor(tB[:, gb], E2Bp[:, gb], Bp[j - 1][:, gb],
                                        ALU.mult)
                if j == 1:
                    gs = ds(g * GB, 2 * SH)
                    gc = ds(g * GB + 2 * SH, 2 * SH)
                    nc.vector.tensor_tensor(Bp[1][:, gs], tB[:, gs],
                                            Bp[0][:, gs], ALU.add)
                    nc.vector.tensor_tensor(Bp[1][:, gc], tB[:, gc],
                                            Bp[0][:, gc], ALU.subtract)
                else:
                    nc.vector.tensor_tensor(Bp[j][:, gb], tB[:, gb],
                                            Bp[j - 2][:, gb], ALU.subtract)
            # A pair step; Pool takes only j==2 (latency: Pool ops are slow)
            engA = nc.gpsimd if j == 2 else nc.vector
            engA.tensor_tensor(tA, E2Ap, Ar[j - 1], ALU.mult)
            if j == 1:
                nc.vector.tensor_tensor(Ar[1][:, 0:AT], tA[:, 0:AT],
                                        Ar[0][:, 0:AT], ALU.add)
                nc.vector.tensor_tensor(Ar[1][:, AT:2 * AT], tA[:, AT:2 * AT],
                                        Ar[0][:, AT:2 * AT], ALU.subtract)
            else:
                engA.tensor_tensor(Ar[j], tA, Ar[j - 2], ALU.subtract)
            engC = nc.gpsimd if j % 2 == 0 else nc.vector
            engC.tensor_scalar_mul(Ap[j], Ar[j], CS[j])
            align_mm(j, False, j == J - 1)

        # ---- epilogue: p = exp(align), sig, V = p16 @ M ------------------
        p32 = P.tile([T, SH], F32, name="p32", tag="p32")
        sig_sb = P.tile([T, 1], F32, name="sig_sb", tag="sig_sb")
        nc.scalar.activation(p32, align_ps, Exp, accum_out=sig_sb[:, 0:1])
        nc.sync.dma_start(out=sig.ap(), in_=sig_sb)
        nc.sync.dma_start(out=p_out.ap(), in_=p32)
        p16 = P.tile([T, SH], F16, name="p16", tag="p16")
        nc.vector.tensor_copy(p16, p32)

        pT_ps = ep_pool.tile([128, 2 * T], F16, name="pT_ps", tag="ep")
        for i in range(2):
            nc.tensor.transpose(pT_ps[:, ts(i, T)], p16[:, ts(i, 128)],
                                ident[0:T, 0:T])
        pT_sb = P.tile([128, 2 * T], F16, name="pT_sb", tag="pT_sb")
        nc.vector.tensor_copy(pT_sb, pT_ps)

        V_ps = ep_pool.tile([T, D], F32, name="V_ps", tag="ep")
        for i in range(2):
            nc.tensor.matmul(V_ps, lhsT=pT_sb[:, ts(i, T)],
                             rhs=M_sb[:, ts(i, D)],
                             start=(i == 0), stop=(i == 1))
        V_sb = P.tile([T, D], F32, name="V_sb", tag="V_sb")
        nc.vector.tensor_copy(V_sb, V_ps)
        nc.sync.dma_start(out=V_out.ap(), in_=V_sb)


def get_nc():
    if "nc" not in _NC_CACHE:
        _NC_CACHE["nc"] = _build_nc()
    return _NC_CACHE["nc"]


def _swz(mT):
    # [rows, F] -> [128, C*F]: partition p holds chunks c at free c*F
    rows, F = mT.shape
    C = rows // 128
    return np.ascontiguousarray(
        mT.reshape(C, 128, F).transpose(1, 0, 2).reshape(128, C * F)
    ).astype(np.float16)


def make_in_maps(inp, context, Wq, bq, Wc, v, Wout, bout):
    inp = np.asarray(inp, np.float32)
    context = np.asarray(context, np.float32)
    wqT = _swz(np.asarray(Wq, np.float32).T)
    wcT = _swz(np.asarray(Wc, np.float32).T)
    woutT = _swz(np.asarray(Wout, np.float32).T)
    bq = np.ascontiguousarray(
        np.asarray(bq, np.float32).reshape(NCH, 128).T)
    v = np.ascontiguousarray(
        np.asarray(v, np.float32).reshape(NCH, 128).T)
    bout = np.asarray(bout, np.float32)
    in_maps = []
    for c in range(N_CORES):
        b, sh = divmod(c, 2)
        in_maps.append({
            "ctxT": _swz(context[b].T[:, sh * SH:(sh + 1) * SH]),
            "wcT": wcT,
            "wqT": wqT,
            "inpT": _swz(inp[b].T),
            "woutT": woutT,
            "bq": bq,
            "v": v,
            "bout": bout,
        })
    return in_maps


def run_on_device(in_maps, **kwargs):
    nc = get_nc()
    return run_bass_kernel_spmd(nc, in_maps, core_ids=list(range(N_CORES)), **kwargs)


def kernel(inp, context, Wq, bq, Wc, v, Wout, bout):
    in_maps = make_in_maps(inp, context, Wq, bq, Wc, v, Wout, bout)
    res = run_on_device(in_maps)
    attn = np.empty((B, T, D), np.float32)
    align = np.empty((B, T, S), np.float32)
    for b in range(B):
        r0 = res.results[2 * b]
        r1 = res.results[2 * b + 1]
        stot = r0["sig"] + r1["sig"]  # (T,1)
        attn[b] = (r0["V_out"] + r1["V_out"]) / stot + r0["I_out"]
        align[b, :, :SH] = r0["p_out"] / stot
        align[b, :, SH:] = r1["p_out"] / stot
    return attn, align


# revision 29
# speedup vs baseline: 1.1617x; 1.1617x over previous
"""Bahdanau (additive) attention kernel for Trainium2, 8 NeuronCores.

Problem shapes: inp (B=4, T=128, D=512), context (B=4, S=512, D=512).
  wq   = inp @ Wq.T + bq                      (B,T,D)
  uh   = context @ Wc.T                       (B,S,D)
  align= einsum('btsd,d->bts', tanh(wq[:,:,None,:]+uh[:,None,:,:]), v)
  a    = softmax(align, -1)                   (B,T,S)
  c    = einsum('bts,bsd->btd', a, context)
  attn = concat([c, inp], -1) @ Wout.T + bout (B,T,D)
Returns (attn, a).

Algorithm: the O(T*S*D) tanh stream is replaced by a separable sinusoid
expansion.  tanh(x) ~ sum_j c_j sin(n_j w x), odd harmonics n_j = 2j+1
(the Neumann-reflected periodic extension of tanh on [-L,L] has only
odd terms).  With a = wq, b = uh:

  align[t,s] ~= sum_{d,j} [c_j sin(n_j w a)] [v_d cos(n_j w b)]
              + [c_j cos(n_j w a)] [v_d sin(n_j w b)]

i.e. one PE matmul with contraction (d, j, phase) = 2*J*D plus
O((T+S)*D*J) trig features.  Feature generation: ACT computes the
n=1 seeds (sin at w/2 and w; cos and E2 = 2cos(2w x) via Square so
every ACT argument stays inside the table range [-pi,pi]); higher
harmonics come from fp16 Chebyshev ladders
  X_{n+2} = E2 * X_n - X_{n-2}
run as paired [sin||cos] tensor_tensor ops on DVE (B side, split in
two chunk-groups so the ladder starts as soon as the first half of the
seeds lands) and DVE+Pool (A side).  v_d rides the (linear) B ladder
seeds; c_j is applied to the raw A harmonics as one wide tensor_scalar
per feature.  All weight/activation operands are pre-swizzled on the
host to [128, C*F] so every DMA is a plain contiguous 2D transfer.

Sharding: 8 cores = (batch b, source-half sh); each core computes its
[T=128, SH=256] block of unnormalized p = exp(align), partial row sums
sig, partial output V = p16 @ M (M = ctx_half @ WoutC), and
I = inp@WoutI + bout.  The host finishes the cross-shard reduction at
gather time: attn = (V0+V1)/(sig0+sig1) + I, align = p/(sig0+sig1).
"""

import numpy as np

import concourse.bacc as bacc
import concourse.tile as tile
from concourse import mybir
from concourse.bass import ds, ts
from concourse.bass_utils import run_bass_kernel_spmd
from concourse.masks import make_identity

F32 = mybir.dt.float32
F16 = mybir.dt.float16
ALU = mybir.AluOpType

B, T, S, D = 4, 128, 512, 512
SH = S // 2  # source positions per core
N_CORES = 8
NCH = D // 128  # partition chunks of the model dim

J = 6
L = 5.8
OM = float(np.pi / (2 * L))
CS = [1.23639529, 0.32902321, 0.13184508, 0.05500646,
      0.02290538, 0.00915409][:J]

_NC_CACHE = {}


def _build_nc():
    nc = bacc.Bacc("TRN2", target_bir_lowering=False, debug=False, num_devices=N_CORES)

    ctxT = nc.dram_tensor("ctxT", [128, NCH * SH], F16, kind="ExternalInput")
    wcT = nc.dram_tensor("wcT", [128, NCH * D], F16, kind="ExternalInput")
    wqT = nc.dram_tensor("wqT", [128, NCH * D], F16, kind="ExternalInput")
    inpT = nc.dram_tensor("inpT", [128, NCH * T], F16, kind="ExternalInput")
    woutT = nc.dram_tensor("woutT", [128, 2 * NCH * D], F16, kind="ExternalInput")
    bq = nc.dram_tensor("bq", [128, NCH], F32, kind="ExternalInput")
    v = nc.dram_tensor("v", [128, NCH], F32, kind="ExternalInput")
    bout = nc.dram_tensor("bout", [D], F32, kind="ExternalInput")
    p_out = nc.dram_tensor("p_out", [T, SH], F32, kind="ExternalOutput")
    sig = nc.dram_tensor("sig", [T, 1], F32, kind="ExternalOutput")
    V_out = nc.dram_tensor("V_out", [T, D], F32, kind="ExternalOutput")
    I_out = nc.dram_tensor("I_out", [T, D], F32, kind="ExternalOutput")

    with tile.TileContext(nc) as tc:
        _emit(nc, tc, ctxT, wcT, wqT, inpT, woutT, bq, v, bout,
              p_out, sig, V_out, I_out)
    nc.compile()
    return nc


def _emit(nc, tc, ctxT, wcT, wqT, inpT, woutT, bq, v, bout,
          p_out, sig, V_out, I_out):
    Sin = mybir.ActivationFunctionType.Sin
    Sq = mybir.ActivationFunctionType.Square
    Exp = mybir.ActivationFunctionType.Exp
    AT = NCH * T    # 512: A-side wide free size
    BT = NCH * SH   # 1024: B-side wide free size
    with (
        tc.tile_pool(name="persist", bufs=1) as P,
        tc.tile_pool(name="uh_ps", bufs=1, space="PSUM") as uh_pool,
        tc.tile_pool(name="wq_ps", bufs=1, space="PSUM") as wq_pool,
        tc.tile_pool(name="al_ps", bufs=1, space="PSUM") as al_pool,
        tc.tile_pool(name="ep_ps", bufs=2, space="PSUM") as ep_pool,
    ):
        # ---- t=0: table preload, PE warmup, DMAs -------------------------
        dumt = P.tile([1, 16], F16, name="dumt", tag="dumt")
        nc.vector.memset(dumt, 0.0)
        dumo = P.tile([1, 16], F16, name="dumo", tag="dumo")
        # first ACT op: loads the trig table set while DMAs run
        nc.scalar.activation(dumo, dumt, Sin)

        warm_sb = P.tile([128, SH], F16, name="warm_sb", tag="warm_sb")
        nc.vector.memset(warm_sb, 0.0)
        warm_ps = ep_pool.tile([128, SH], F32, name="warm_ps", tag="ep")
        for r in range(8):
            nc.tensor.matmul(warm_ps[0:64, :], lhsT=warm_sb[:, 0:64], rhs=warm_sb,
                             start=(r == 0), stop=(r == 7))

        def filler(n=1):
            # keep the PE HAM clock warm during feature-ladder stalls
            for r in range(n):
                nc.tensor.matmul(warm_ps[0:64, 0:128], lhsT=warm_sb[:, 0:64],
                                 rhs=warm_sb[:, 0:128], start=True, stop=True)

        def load_wide(name, dram, engine=None):
            # host pre-swizzles to [128, C*F]: plain contiguous 2D DMA
            t = P.tile([128, dram.shape[1]], F16, name=name, tag=name)
            eng = engine or nc.sync
            eng.dma_start(out=t, in_=dram.ap())
            return t

        v_sb = P.tile([128, NCH], F32, name="v_sb", tag="v_sb")
        nc.sync.dma_start(out=v_sb, in_=v.ap())
        bq_sb = P.tile([128, NCH], F32, name="bq_sb", tag="bq_sb")
        nc.sync.dma_start(out=bq_sb, in_=bq.ap())
        ctxT_all = load_wide("ctxT_all", ctxT)          # [128, 4*SH]
        wcT_all = load_wide("wcT_all", wcT, nc.gpsimd)  # [128, 4*D]
        wqT_all = load_wide("wqT_all", wqT, nc.scalar)  # [128, 4*D]
        inpT_all = load_wide("inpT_all", inpT, nc.scalar)  # [128, 4*T]

        ctxT_sb = [ctxT_all[:, ds(SH * i, SH)] for i in range(NCH)]
        wcT_sb = [wcT_all[:, ds(D * i, D)] for i in range(NCH)]
        wqT_sb = [wqT_all[:, ds(D * i, D)] for i in range(NCH)]
        inpT_sb = [inpT_all[:, ds(T * i, T)] for i in range(NCH)]

        ident = P.tile([128, 128], F16, name="ident", tag="ident")
        make_identity(nc, ident)
        ones_sb = P.tile([1, T], F16, name="ones_sb", tag="ones_sb")
        nc.gpsimd.memset(ones_sb, 1.0)
        # sin(w (wq + bq)) = Sin(scale=w, bias=w*bq) on the raw wq PSUM
        bqw2 = P.tile([128, NCH], F32, name="bqw2", tag="bqw2")
        nc.vector.tensor_scalar_mul(bqw2, bq_sb, OM / 2)
        bqw = P.tile([128, NCH], F32, name="bqw", tag="bqw")
        nc.vector.tensor_scalar_mul(bqw, bq_sb, OM)

        # ---- B-side: uh matmuls + per-chunk seed pipeline ----------------
        # seeds: sh = sin(w/2 uh), s1 = sin(w uh) straight from PSUM;
        # cos1 = 1-2 sh^2, E2 = 2-4 s1^2 (Square + tensor_scalar).
        uh_wide = uh_pool.tile([128, BT], F32, name="uh_wide", tag="uh")
        uh_ps = [uh_wide[:, ts(k, SH)] for k in range(NCH)]
        shB = P.tile([128, BT], F16, name="shB", tag="shB")
        s1Br = P.tile([128, BT], F16, name="s1Br", tag="s1Br")
        qB = P.tile([128, BT], F16, name="qB", tag="qB")
        qB2 = P.tile([128, BT], F16, name="qB2", tag="qB2")
        c1Br = P.tile([128, BT], F16, name="c1Br", tag="c1Br")
        E2Bp = P.tile([128, 2 * BT], F16, name="E2Bp", tag="E2Bp")
        # paired [sin || cos] B feature tiles, v-scaled
        Bp = [P.tile([128, 2 * BT], F16, name=f"Bp{j}", tag=f"Bp{j}")
              for j in range(J)]

        # Bp layout is group-major: group g (chunks 2g,2g+1) occupies
        # [g*BT : g*BT+BT] as [sin(512) || cos(512)]; E2Bp matches with the
        # E2 values duplicated into both halves of each group.
        GB = BT  # 1024: bytes of one [sin||cos] group block
        def bsin(k):
            return (k // 2) * GB + (k % 2) * SH
        def bcos(k):
            return (k // 2) * GB + 2 * SH + (k % 2) * SH
        for k in range(NCH):
            for j in range(NCH):
                nc.tensor.matmul(uh_ps[k], lhsT=wcT_sb[j][:, ts(k, 128)],
                                 rhs=ctxT_sb[j], start=(j == 0), stop=(j == NCH - 1))
            cc = ts(k, SH)
            nc.scalar.activation(shB[:, cc], uh_ps[k], Sin, scale=OM / 2)
            nc.scalar.activation(s1Br[:, cc], uh_ps[k], Sin, scale=OM)
            if k % 2 == 1:
                g = ds((k - 1) * SH, 2 * SH)
                gb = (k - 1) // 2 * GB
                nc.scalar.activation(qB[:, g], shB[:, g], Sq)
                nc.scalar.activation(qB2[:, g], s1Br[:, g], Sq)
                nc.vector.tensor_scalar(c1Br[:, g], qB[:, g], -2.0, 1.0,
                                        ALU.mult, ALU.add)
                nc.vector.tensor_scalar(E2Bp[:, ds(gb, 2 * SH)], qB2[:, g],
                                        -4.0, 2.0, ALU.mult, ALU.add)
                nc.vector.tensor_scalar(E2Bp[:, ds(gb + 2 * SH, 2 * SH)],
                                        qB2[:, g], -4.0, 2.0, ALU.mult, ALU.add)
                for kk in (k - 1, k):
                    nc.gpsimd.tensor_scalar_mul(Bp[0][:, ds(bcos(kk), SH)],
                                                c1Br[:, ts(kk, SH)],
                                                v_sb[:, kk:kk + 1])
            nc.gpsimd.tensor_scalar_mul(Bp[0][:, ds(bsin(k), SH)], s1Br[:, cc],
                                        v_sb[:, k:k + 1])

        # ---- A-side: wq matmuls + seeds (PSUM + bias trick) --------------
        wq_wide = wq_pool.tile([128, AT], F32, name="wq_wide", tag="wq")
        wq_ps = [wq_wide[:, ts(k, T)] for k in range(NCH)]
        shA = P.tile([128, AT], F16, name="shA", tag="shA")
        s1Ar = P.tile([128, AT], F16, name="s1Ar", tag="s1Ar")
        qA = P.tile([128, AT], F16, name="qA", tag="qA")
        qA2 = P.tile([128, AT], F16, name="qA2", tag="qA2")
        c1Ar = P.tile([128, AT], F16, name="c1Ar", tag="c1Ar")
        # E2A duplicated [E2A || E2A] for paired A steps
        E2Ap = P.tile([128, 2 * AT], F16, name="E2Ap", tag="E2Ap")
        for k in range(NCH):
            for j in range(NCH):
                nc.tensor.matmul(wq_ps[k], lhsT=wqT_sb[j][:, ts(k, 128)],
                                 rhs=inpT_sb[j], start=(j == 0), stop=(j == NCH - 1))
            cc = ts(k, T)
            nc.scalar.activation(shA[:, cc], wq_ps[k], Sin, scale=OM / 2,
                                 bias=bqw2[:, k:k + 1])
            nc.scalar.activation(s1Ar[:, cc], wq_ps[k], Sin, scale=OM,
                                 bias=bqw[:, k:k + 1])
            nc.scalar.activation(qA[:, cc], shA[:, cc], Sq)
            nc.scalar.activation(qA2[:, cc], s1Ar[:, cc], Sq)
            nc.vector.tensor_scalar(c1Ar[:, cc], qA[:, cc], -2.0, 1.0,
                                    ALU.mult, ALU.add)
            nc.vector.tensor_scalar(E2Ap[:, cc], qA2[:, cc], -4.0, 2.0,
                                    ALU.mult, ALU.add)
            nc.vector.tensor_scalar(E2Ap[:, ds(AT + k * T, T)], qA2[:, cc],
                                    -4.0, 2.0, ALU.mult, ALU.add)

        # all Sin work is done once the A seeds above retire; preload the
        # exp table now (data-dependent on the last Square so the scheduler
        # keeps it after every Sin) so the softmax tail pays no table load
        nc.scalar.activation(dumo, qA2[0:1, AT - 16:AT], Exp)

        # raw paired A harmonic chain + c_j-scaled feature tiles
        Ar = [P.tile([128, 2 * AT], F16, name=f"Ar{j}", tag=f"Ar{j}")
              for j in range(J)]
        Ap = [P.tile([128, 2 * AT], F16, name=f"Ap{j}", tag=f"Ap{j}")
              for j in range(J)]
        nc.vector.tensor_copy(Ar[0][:, 0:AT], s1Ar)
        nc.vector.tensor_copy(Ar[0][:, AT:2 * AT], c1Ar)
        nc.vector.tensor_scalar_mul(Ap[0], Ar[0], CS[0])

        # ---- epilogue operands (loaded/computed mid-stream) --------------
        woutT_all = load_wide("woutT_all", woutT, nc.scalar)
        woutT_sb = [woutT_all[:, ds(D * i, D)] for i in range(2 * NCH)]
        bout_f32 = P.tile([1, D], F32, name="bout_f32", tag="bout_f32")
        nc.scalar.dma_start(out=bout_f32, in_=bout.ap().rearrange("(o f) -> o f", o=1))
        bout_sb = P.tile([1, D], F16, name="bout_sb", tag="bout_sb")
        nc.gpsimd.tensor_copy(bout_sb, bout_f32)

        align_ps = al_pool.tile([T, SH], F32, name="align", tag="align")

        def align_mm(j, start, stop):
            # align += As_j^T Bc_j + Ac_j^T Bs_j over the 4 d-chunks
            for k in range(NCH):
                nc.tensor.matmul(align_ps, lhsT=Ap[j][:, ts(k, T)],
                                 rhs=Bp[j][:, ds(bcos(k), SH)],
                                 start=start and k == 0, stop=False)
            for k in range(NCH):
                nc.tensor.matmul(align_ps, lhsT=Ap[j][:, ds(AT + k * T, T)],
                                 rhs=Bp[j][:, ds(bsin(k), SH)],
                                 start=False, stop=stop and k == NCH - 1)

        M_sb = P.tile([128, 2 * D], F16, name="M_sb", tag="M_sb")

        def emit_M_chunk(sc):
            # M[s, e] = sum_f ctx[s, f] Wout_c[e, f]
            ps = ep_pool.tile([128, D], F32, name=f"M{sc}", tag="ep")
            for j in range(NCH):
                nc.tensor.matmul(ps, lhsT=ctxT_all[:, ds(SH * j + 128 * sc, 128)],
                                 rhs=woutT_sb[j], start=(j == 0), stop=(j == NCH - 1))
            nc.scalar.copy(M_sb[:, ts(sc, D)], ps)

        I_sb = P.tile([T, D], F32, name="I_sb", tag="I_sb")

        def emit_I():
            ps = ep_pool.tile([T, D], F32, name="I_ps", tag="ep")
            nc.tensor.matmul(ps, lhsT=ones_sb[:, 0:T], rhs=bout_sb,
                             start=True, stop=False)
            for f in range(NCH):
                nc.tensor.matmul(ps, lhsT=inpT_sb[f], rhs=woutT_sb[NCH + f],
                                 start=False, stop=(f == NCH - 1))
            nc.scalar.copy(I_sb, ps)
            nc.scalar.dma_start(out=I_out.ap(), in_=I_sb)

        # ---- ladders + align accumulation --------------------------------
        # B (v-carried, unscaled c): X_{j} = E2B*X_{j-1} -/+ X_{j-2}
        # A (c-folded): F_j = Ea_j*F_{j-1} + (-beta_j) F_{j-2} via STT
        tmpB = [P.tile([128, 2 * BT], F16, name=f"tmpB{i}", tag=f"tmpB{i}")
                for i in range(2)]
        tmpA = [P.tile([128, 2 * AT], F16, name=f"tmpA{i}", tag=f"tmpA{i}")
                for i in range(2)]

        emit_M_chunk(0)
        emit_M_chunk(1)
        emit_I()
        align_mm(0, True, False)

        for j in range(1, J):
            tB = tmpB[j % 2]
            tA = tmpA[j % 2]
            for g in range(2):
                gb = ds(g * GB, GB)
                nc.vector.tensor_tensor(tB[:, gb], E2Bp[:, gb], Bp[j - 1][:, gb],
                                        ALU.mult)
                if j == 1:
                    gs = ds(g * GB, 2 * SH)
                    gc = ds(g * GB + 2 * SH, 2 * SH)
                    nc.vector.tensor_tensor(Bp[1][:, gs], tB[:, gs],
                                            Bp[0][:, gs], ALU.add)
                    nc.vector.tensor_tensor(Bp[1][:, gc], tB[:, gc],
                                            Bp[0][:, gc], ALU.subtract)
                else:
                    nc.vector.tensor_tensor(Bp[j][:, gb], tB[:, gb],
                                            Bp[j - 2][:, gb], ALU.subtract)
            # A pair step; Pool takes only j==2 (latency: Pool ops are slow)
            engA = nc.gpsimd if j == 2 else nc.vector
            engA.tensor_tensor(tA, E2Ap, Ar[j - 1], ALU.mult)
            if j == 1:
                nc.vector.tensor_tensor(Ar[1][:, 0:AT], tA[:, 0:AT],
                                        Ar[0][:, 0:AT], ALU.add)
                nc.vector.tensor_tensor(Ar[1][:, AT:2 * AT], tA[:, AT:2 * AT],
                                        Ar[0][:, AT:2 * AT], ALU.subtract)
            else:
                engA.tensor_tensor(Ar[j], tA, Ar[j - 2], ALU.subtract)
            engC = nc.gpsimd if j % 2 == 0 else nc.vector
            engC.tensor_scalar_mul(Ap[j], Ar[j], CS[j])
            align_mm(j, False, j == J - 1)

        # ---- epilogue: p = exp(align), sig, V = p16 @ M ------------------
        p32 = P.tile([T, SH], F32, name="p32", tag="p32")
        sig_sb = P.tile([T, 1], F32, name="sig_sb", tag="sig_sb")
        nc.scalar.activation(p32, align_ps, Exp, accum_out=sig_sb[:, 0:1])
        nc.sync.dma_start(out=sig.ap(), in_=sig_sb)
        nc.sync.dma_start(out=p_out.ap(), in_=p32)
        p16 = P.tile([T, SH], F16, name="p16", tag="p16")
        nc.vector.tensor_copy(p16, p32)

        pT_ps = ep_pool.tile([128, 2 * T], F16, name="pT_ps", tag="ep")
        for i in range(2):
            nc.tensor.transpose(pT_ps[:, ts(i, T)], p16[:, ts(i, 128)],
                                ident[0:T, 0:T])
        pT_sb = P.tile([128, 2 * T], F16, name="pT_sb", tag="pT_sb")
        nc.vector.tensor_copy(pT_sb, pT_ps)

        V_ps = ep_pool.tile([T, D], F32, name="V_ps", tag="ep")
        for i in range(2):
            nc.tensor.matmul(V_ps, lhsT=pT_sb[:, ts(i, T)],
                             rhs=M_sb[:, ts(i, D)],
                             start=(i == 0), stop=(i == 1))
        V_sb = P.tile([T, D], F32, name="V_sb", tag="V_sb")
        nc.vector.tensor_copy(V_sb, V_ps)
        nc.sync.dma_start(out=V_out.ap(), in_=V_sb)


def get_nc():
    if "nc" not in _NC_CACHE:
        _NC_CACHE["nc"] = _build_nc()
    return _NC_CACHE["nc"]


def _swz(mT):
    # [rows, F] -> [128, C*F]: partition p holds chunks c at free c*F
    rows, F = mT.shape
    C = rows // 128
    return np.ascontiguousarray(
        mT.reshape(C, 128, F).transpose(1, 0, 2).reshape(128, C * F)
    ).astype(np.float16)


def make_in_maps(inp, context, Wq, bq, Wc, v, Wout, bout):
    inp = np.asarray(inp, np.float32)
    context = np.asarray(context, np.float32)
    wqT = _swz(np.asarray(Wq, np.float32).T)
    wcT = _swz(np.asarray(Wc, np.float32).T)
    woutT = _swz(np.asarray(Wout, np.float32).T)
    bq = np.ascontiguousarray(
        np.asarray(bq, np.float32).reshape(NCH, 128).T)
    v = np.ascontiguousarray(
        np.asarray(v, np.float32).reshape(NCH, 128).T)
    bout = np.asarray(bout, np.float32)
    in_maps = []
    for c in range(N_CORES):
        b, sh = divmod(c, 2)
        in_maps.append({
            "ctxT": _swz(context[b].T[:, sh * SH:(sh + 1) * SH]),
            "wcT": wcT,
            "wqT": wqT,
            "inpT": _swz(inp[b].T),
            "woutT": woutT,
            "bq": bq,
            "v": v,
            "bout": bout,
        })
    return in_maps


def run_on_device(in_maps, **kwargs):
    nc = get_nc()
    return run_bass_kernel_spmd(nc, in_maps, core_ids=list(range(N_CORES)), **kwargs)


def kernel(inp, context, Wq, bq, Wc, v, Wout, bout):
    in_maps = make_in_maps(inp, context, Wq, bq, Wc, v, Wout, bout)
    res = run_on_device(in_maps)
    attn = np.empty((B, T, D), np.float32)
    align = np.empty((B, T, S), np.float32)
    for b in range(B):
        r0 = res.results[2 * b]
        r1 = res.results[2 * b + 1]
        stot = r0["sig"] + r1["sig"]  # (T,1)
        attn[b] = (r0["V_out"] + r1["V_out"]) / stot + r0["I_out"]
        align[b, :, :SH] = r0["p_out"] / stot
        align[b, :, SH:] = r1["p_out"] / stot
    return attn, align
